# revision 16
# baseline (speedup 1.0000x reference)
"""GQA causal attention block (B=2, L=2048, d_model=2048, 32 Q heads / 8 KV heads)
on 8 TRN2 NeuronCores.

Sharding: 8-way tensor parallel over heads, batch-paired. Core c owns q-heads
[4c, 4c+4) and kv-head c FOR BOTH batches.

Layouts:
  - qT (bf16): 4 tiles [128, L] = head-pair x batch: tile (t, b) holds local
    heads t (partitions 0:64) and t+2 (64:128) of batch b, feature-major.
  - kT (bf16): per batch a [128, L] tile with the kv head DUPLICATED in both
    partition halves, so heads 2,3 read lhsT/rhs at matching base 64.
  - v_aug (bf16): per key-block [128, 130]: cols 0:65 = batch0 (v | ones),
    65:130 = batch1 (v | ones).

Attention runs per head-pair (t, t+2 share a partition base, so consecutive
scores matmuls reuse the same kT stationary operand - the K=64 weight load
cannot be hidden otherwise). Scores^T -> exp (ScalarE, psum->bf16, causal
block masking) -> AV against V-with-ones-column (emits attn^T + softmax
denominator). Two AllToAll collectives (heads {0,1} then {2,3}) overlap with
the second half's attention and the first half's o_proj. o_proj (fp32r,
full Wo) accumulates even f-blocks after A2A#1 and odd after A2A#2; division
by the softmax denominator is fused in front via reciprocal + partition-
broadcast DMA + DVE multiply. The host just stacks output rows.
"""

import os
import sys
import math

os.environ.setdefault("MYCRO_LOCAL_CACHE", "1")
for _p in ("/opt/trn_rl_repo",):
    if os.path.isdir(_p) and _p not in sys.path:
        sys.path.insert(0, _p)

import numpy as np

import concourse.bass as bass
import concourse.bacc as bacc
import concourse.mybir as mybir
import concourse.tile as tile
from concourse.bass_utils import run_bass_kernel_spmd
from concourse.masks import make_identity

F32 = mybir.dt.float32
F32R = mybir.dt.float32r
BF16 = mybir.dt.bfloat16
Exp = mybir.ActivationFunctionType.Exp

D = 2048          # d_model
L = 2048          # sequence length
DH = 64           # head dim
B = 2             # batch
NCORES = 8
NH_L = 4          # local q heads per core (per batch)
QF = NH_L * DH    # 256 local q features per batch
LC1 = 256         # phase-1 l-chunk (moving dim)
NLC1 = L // LC1   # 8
LC = 512          # attention l-tile
NLC = L // LC     # 4
NB = L // 128     # 16 key blocks of 128
SH = 130          # rows per A2A half-shard (2 heads x 64 + 2 denoms)
SCALE = 1.0 / math.sqrt(DH)

_CACHE = {}


def _mmr(nc, out, lhsT, rhs, **kw):
    nc.tensor.matmul(out, lhsT, rhs, **kw)


def _build_nc():
    nc = bacc.Bacc(
        "TRN2",
        target_bir_lowering=False,
        debug=False,
        enable_asserts=False,
        num_devices=NCORES,
    )
    xT0 = nc.dram_tensor("xT0", [D, L], F32, kind="ExternalInput")
    xT1 = nc.dram_tensor("xT1", [D, L], F32, kind="ExternalInput")
    wqT = nc.dram_tensor("wqT", [D, QF], F32, kind="ExternalInput")
    wkT = nc.dram_tensor("wkT", [D, DH], F32, kind="ExternalInput")
    wvT = nc.dram_tensor("wvT", [D, DH], F32, kind="ExternalInput")
    woT = nc.dram_tensor("woT", [D, D], F32, kind="ExternalInput")
    y = nc.dram_tensor("y", [LC, D], F32, kind="ExternalOutput")

    with tile.TileContext(nc) as tc:
        with tc.tile_pool(name="dram", bufs=1, space="DRAM") as dram:
            # one bounce pair per head-half (heads {0,1} / {2,3})
            bins = [
                dram.tile([NCORES * SH, LC], F32, name=f"bin{h}") for h in range(2)
            ]
            bouts = [
                dram.tile([NCORES * SH, LC], F32, name=f"bout{h}") for h in range(2)
            ]
            rdram = dram.tile([32, 512], F32, name="rdram")

            with tc.tile_pool(name="const", bufs=1) as const:
                ident = const.tile([128, 128], F32, name="ident")
                make_identity(nc, ident)

                with tc.tile_pool(name="pers", bufs=1) as pers:
                    # qT[t][b]: heads t | t+2 of batch b
                    qT = [
                        [
                            pers.tile([128, L], BF16, name=f"qT{t}{b}")
                            for b in range(2)
                        ]
                        for t in range(2)
                    ]
                    kT = [pers.tile([128, L], BF16, name=f"kT{b}") for b in range(2)]
                    vaug = pers.tile([128, NB * 130], BF16, name="vaug")
                    va = vaug.rearrange("p (b c) -> p b c", c=130)
                    nc.gpsimd.memset(va[:, :, 64:65], 1.0)
                    nc.gpsimd.memset(va[:, :, 129:130], 1.0)

                    # wo streamed early: pool opened before phase 2 so its
                    # DMAs overlap attention compute
                    with tc.tile_pool(name="wo", bufs=1) as wop:
                        wo_t = []
                        for dc in range(4):
                            w = wop.tile(
                                [128, 16 * 512], F32R, name=f"wo{dc}", tag="wo"
                            )
                            nc.gpsimd.dma_start(
                                w.rearrange("p (b d) -> p b d", d=512),
                                woT[:, dc * 512 : (dc + 1) * 512].rearrange(
                                    "(b p) d -> p b d", p=128
                                ),
                            )
                            wo_t.append(w)

                        _phase1_qkv(
                            nc, tc, xT0, xT1, wqT, wkT, wvT, qT, kT, va, ident
                        )
                        with (
                            tc.tile_pool(name="p2s", bufs=1, space="PSUM") as scp,
                            tc.tile_pool(name="p2o", bufs=1, space="PSUM") as ovp,
                            tc.tile_pool(name="pbuf", bufs=1) as pbp,
                            tc.tile_pool(name="stg", bufs=2) as stp,
                        ):
                            for half in range(2):
                                _attn_half(
                                    nc, tc, qT, kT, va, bins[half], half,
                                    scp, ovp, pbp, stp,
                                )
                                nc.gpsimd.collective_compute(
                                    "AllToAll",
                                    mybir.AluOpType.bypass,
                                    ins=[bins[half].opt()],
                                    outs=[bouts[half].opt()],
                                    replica_groups=[list(range(NCORES))],
                                )
                        _phase4_oproj(nc, tc, bouts, wo_t, rdram, y)
    nc.finalize()
    return nc


def _phase1_qkv(nc, tc, xT0, xT1, wqT, wkT, wvT, qT, kT, va, ident):
    """Projections. q: one M=128 fp32r matmul per (head-pair, batch, db).
    k/v: M=64 per batch at psum base 0; the partition-64 halves of kT are
    filled via a bf16 staging tile + SBUF->SBUF DMA."""
    with (
        tc.tile_pool(name="w1", bufs=1) as wpool,
        tc.tile_pool(name="xc", bufs=2) as xpool,
        tc.tile_pool(name="vt", bufs=2) as vtpool,
        tc.tile_pool(name="p1", bufs=1, space="PSUM") as p1,
    ):
        wq_sb = wpool.tile([128, 16 * QF], F32R, name="wq_sb")
        wk_sb = wpool.tile([128, 16 * DH], F32R, name="wk_sb")
        wv_sb = wpool.tile([128, 16 * DH], F32R, name="wv_sb")
        for w_sb, w_dram, fw in ((wq_sb, wqT, QF), (wk_sb, wkT, DH), (wv_sb, wvT, DH)):
            nc.gpsimd.dma_start(
                w_sb.rearrange("p (b f) -> p b f", f=fw),
                w_dram.rearrange("(b p) f -> p b f", p=128),
            )

        for lc in range(NLC1):
            x0 = xpool.tile([128, 16 * LC1], F32R, name="x0", tag="x0")
            x1 = xpool.tile([128, 16 * LC1], F32R, name="x1", tag="x1")
            for xt, xdram in ((x0, xT0), (x1, xT1)):
                nc.gpsimd.dma_start(
                    xt.rearrange("p (b l) -> p b l", l=LC1),
                    xdram[:, lc * LC1 : (lc + 1) * LC1].rearrange(
                        "(b p) l -> p b l", p=128
                    ),
                )
            cols = slice(lc * LC1, (lc + 1) * LC1)
            for grp in range(2):
                aq = [
                    p1.tile([128, LC1], F32, name=f"aq{b}", tag=f"aq{b}")
                    for b in range(2)
                ]
                akv = [
                    p1.tile([64, LC1], F32, name=f"akv{b}", tag=f"akv{b}")
                    for b in range(2)
                ]
                for db in range(16):
                    rx = (
                        x0[:, db * LC1 : (db + 1) * LC1],
                        x1[:, db * LC1 : (db + 1) * LC1],
                    )
                    st = dict(start=(db == 0), stop=(db == 15))
                    wjp = wq_sb[:, db * QF + grp * 128 : db * QF + (grp + 1) * 128]
                    wkv = wk_sb if grp == 0 else wv_sb
                    wb = wkv[:, db * DH : (db + 1) * DH]
                    for b in range(2):
                        _mmr(nc, aq[b][:, :], wjp, rx[b], **st)
                        _mmr(nc, akv[b][:, :], wb, rx[b], **st)
                for b in range(2):
                    nc.scalar.copy(qT[grp][b][:, cols], aq[b][:, :])
                if grp == 0:
                    for b in range(2):
                        nc.scalar.copy(kT[b][0:64, cols], akv[b][:, :])
                        stk = vtpool.tile([64, LC1], BF16, name="stk", tag=f"stk{b}")
                        nc.scalar.copy(stk[:, :], akv[b][:, :])
                        nc.sync.dma_start(kT[b][64:128, cols], stk[:, :])
                else:
                    vt0 = vtpool.tile([64, LC1], F32, name="vt0", tag="vt0")
                    vt1 = vtpool.tile([64, LC1], F32, name="vt1", tag="vt1")
                    nc.scalar.copy(vt0[:, :], akv[0][:, :])
                    nc.scalar.copy(vt1[:, :], akv[1][:, :])
                    for s in range(LC1 // 128):
                        beta = (lc * LC1) // 128 + s
                        tp = p1.tile([128, 128], F32, name="tp", tag="tp", bufs=2)
                        nc.tensor.matmul(
                            tp[:, 0:64],
                            vt0[:, s * 128 : (s + 1) * 128],
                            ident[0:64, 0:64],
                            is_transpose=True,
                        )
                        nc.tensor.matmul(
                            tp[:, 64:128],
                            vt1[:, s * 128 : (s + 1) * 128],
                            ident[0:64, 0:64],
                            is_transpose=True,
                            skip_group_check=True,
                        )
                        nc.scalar.copy(va[:, beta, 0:64], tp[:, 0:64])
                        nc.scalar.copy(va[:, beta, 65:129], tp[:, 64:128])


def _attn_half(nc, tc, qT, kT, va, bin_, half, scp, ovp, pbp, stp):
    """Attention for local heads {half, half+... } i.e. heads j in
    {0,1} (half 0, partition base 0) or {2,3} (half 1, base 64).

    Heads j=half*2.. wait: half 0 -> heads 0,1 (tiles t=0,1 partitions 0:64);
    half 1 -> heads 2,3 (tiles t=0,1 partitions 64:128). Both heads of a half
    share the kT partition base, so their scores matmuls reuse each stationary
    kT block back to back."""
    po = 64 * half
    for tau in range(NLC):
        nb = 4 * tau + 4
        lcols = slice(tau * LC, (tau + 1) * LC)
        # P buffers: [head-in-half u][batch b]
        P = [
            [
                pbp.tile([128, NB * 512], BF16, name=f"p{u}{b}", tag=f"p{u}{b}")
                for b in range(2)
            ]
            for u in range(2)
        ]
        # scores psum: one [128, 1024] (2-bank) tile per head-in-half
        q = [
            [qT[u][b][po : po + 64, lcols] for b in range(2)] for u in range(2)
        ]

        def scores(bslc, kslc0, kslc1=None, expslc=None, dj=0):
            """one merged strip group for all (u, b) streams"""
            for b in range(2):
                k0 = kT[b][po : po + 64, kslc0]
                k1 = kT[b][po : po + 64, kslc1] if kslc1 is not None else None
                for u in range(2):
                    sc = scp.tile(
                        [128, 1024], F32, name=f"sc{u}", tag=f"sc{u}"
                    )
                    nc.tensor.matmul(sc[:, 0:512], k0, q[u][b])
                    if k1 is not None:
                        nc.tensor.matmul(sc[:, 512:1024], k1, q[u][b])
                        nc.scalar.activation(
                            P[u][b][:, bslc], sc[:, 0:1024], Exp
                        )
                    else:
                        if dj > 0:
                            nc.gpsimd.memset(
                                P[u][b][:, expslc.start - dj * 128 : expslc.start],
                                0.0,
                            )
                        nc.scalar.activation(
                            P[u][b][:, expslc], sc[:, dj * 128 : 512], Exp
                        )
                        dg = P[u][b][
                            :, expslc.start : expslc.start + 128
                        ]
                        nc.gpsimd.affine_select(
                            out=dg, in_=dg,
                            compare_op=mybir.AluOpType.is_ge,
                            fill=0.0, base=0,
                            pattern=[[1, 128]], channel_multiplier=-1,
                        )

        for b0 in range(0, 4 * tau, 2):
            scores(
                slice(b0 * 512, (b0 + 2) * 512),
                slice(b0 * 128, (b0 + 1) * 128),
                slice((b0 + 1) * 128, (b0 + 2) * 128),
            )
        for dj in range(4):
            beta = 4 * tau + dj
            scores(
                None,
                slice(beta * 128, (beta + 1) * 128),
                None,
                expslc=slice(beta * 512 + dj * 128, (beta + 1) * 512),
                dj=dj,
            )

        # AV: 4 chains (u x b), interleaved
        ov = [
            [
                ovp.tile([128, 512], F32, name=f"o{u}{b}", tag=f"o{u}{b}")
                for b in range(2)
            ]
            for u in range(2)
        ]
        for bk in range(nb):
            st = dict(start=(bk == 0), stop=(bk == nb - 1))
            for u in range(2):
                for b in range(2):
                    nc.tensor.matmul(
                        ov[u][b][0:65, :],
                        va[:, bk, 65 * b : 65 * b + 65],
                        P[u][b][:, bk * 512 : (bk + 1) * 512],
                        **st,
                    )
        # stage to bounce: shard (4*b + tau): rows 64*u attn, 128+u denom
        for u in range(2):
            st1 = stp.tile([128, 512], F32, name="st1", tag="st1")
            nc.scalar.copy(st1[0:64, :], ov[u][0][0:64, :])
            nc.scalar.copy(st1[64:128, :], ov[u][1][0:64, :])
            ds = stp.tile([128, 1024], F32, name="ds", tag="ds")
            nc.vector.tensor_copy(ds[64:65, 0:512], ov[u][0][64:65, :])
            nc.vector.tensor_copy(ds[64:65, 512:1024], ov[u][1][64:65, :])
            for b in range(2):
                sh = SH * (4 * b + tau)
                nc.sync.dma_start(
                    bin_[sh + 64 * u : sh + 64 * (u + 1), :],
                    st1[64 * b : 64 * (b + 1), :],
                )
                nc.sync.dma_start(
                    bin_[sh + 128 + u : sh + 128 + u + 1, :],
                    ds[64:65, 512 * b : 512 * b + 512],
                )


def _phase4_oproj(nc, tc, bouts, wo_t, rdram, y):
    """Normalize and o_proj. f-block k holds global heads 2k,2k+1 =
    bounce[(2k)%4//2][shard k//2] rows 0:128; even k uses bout0, odd bout1."""
    with (
        tc.tile_pool(name="an", bufs=1, side="right") as anp,
        tc.tile_pool(name="den", bufs=1, side="right") as denp,
        tc.tile_pool(name="ysb", bufs=2, side="right") as yp,
        tc.tile_pool(name="p4y", bufs=4, space="PSUM") as eyp,
    ):
        # reciprocals per half: rdram rows = global head id
        for half in range(2):
            dall = denp.tile([16, 512], F32, name=f"dall{half}", tag="dall", bufs=2)
            for c in range(NCORES):
                nc.sync.dma_start(
                    dall[2 * c : 2 * (c + 1), :],
                    bouts[half][SH * c + 128 : SH * c + 130, :],
                )
            rall = denp.tile([16, 512], F32, name=f"rall{half}", tag="rall", bufs=2)
            nc.vector.reciprocal(rall[:, :], dall[:, :])
            # rdram row layout: 16*half + 2*c + u  (head h = 4c + 2*half + u)
            nc.sync.dma_start(rdram[16 * half : 16 * half + 16, :], rall[:, :])

        ans = [None] * 16
        for k in sorted(range(16), key=lambda k: k % 2):
            half = k % 2
            c = k // 2
            au = anp.tile([128, 512], F32, name=f"au{k}", tag="au", bufs=4)
            nc.sync.dma_start(
                au[:, :], bouts[half][SH * c : SH * c + 128, :]
            )
            # heads 2k, 2k+1 -> rdram rows 16*half + 2*c (+1)
            rA = 16 * half + 2 * c
            dv = anp.tile([128, 512], F32, name="dv", tag="dv", bufs=4)
            nc.sync.dma_start(
                dv[0:64, :], rdram[rA : rA + 1, :].partition_broadcast(64)
            )
            nc.sync.dma_start(
                dv[64:128, :], rdram[rA + 1 : rA + 2, :].partition_broadcast(64)
            )
            an = anp.tile([128, 512], F32R, name=f"an{k}", tag=f"an{k}")
            nc.vector.tensor_mul(an[:, :], au[:, :], dv[:, :])
            ans[k] = an

        korder = sorted(range(16), key=lambda k: k % 2)
        for dc in range(4):
            for m in range(4):
                yps = eyp.tile([128, 512], F32, name="yps", tag="yps")
                for i, k in enumerate(korder):
                    _mmr(
                        nc, yps[:, :],
                        ans[k][:, m * 128 : (m + 1) * 128],
                        wo_t[dc][:, k * 512 : (k + 1) * 512],
                        start=(i == 0), stop=(i == 15),
                    )
                ysb = yp.tile([128, 512], F32, name="ysb", tag="ysb")
                nc.scalar.copy(ysb[:, :], yps[:, :])
                nc.sync.dma_start(
                    y[m * 128 : (m + 1) * 128, dc * 512 : (dc + 1) * 512], ysb[:, :]
                )


def _get_nc():
    if "nc" not in _CACHE:
        _CACHE["nc"] = _build_nc()
    return _CACHE["nc"]


LAST_EXEC_NS = None


def _host_in_maps(x, Wq, Wk, Wv, Wo):
    xT0 = np.ascontiguousarray(x[0].T)
    xT1 = np.ascontiguousarray(x[1].T)
    woT = np.ascontiguousarray(Wo.T)

    in_maps = []
    for c in range(NCORES):
        # local head order in wq columns: pair-tile layout (t | t+2):
        # tile 0 = heads 0,2 ; tile 1 = heads 1,3  (within this core)
        rows = []
        for t in range(2):
            for u in range(2):
                h = QF * c // 64 + t + 2 * u  # 4c + t + 2u
                rows.append(Wq[64 * h : 64 * (h + 1), :])
        # order per 128-block: block grp holds heads (grp, grp+2)
        wq_rows = np.concatenate(
            [rows[0], rows[1], rows[2], rows[3]], axis=0
        )
        wqT_c = np.ascontiguousarray((SCALE * wq_rows).T)
        wkT_c = np.ascontiguousarray(Wk[DH * c : DH * (c + 1), :].T)
        wvT_c = np.ascontiguousarray(Wv[DH * c : DH * (c + 1), :].T)
        in_maps.append(
            {
                "xT0": xT0,
                "xT1": xT1,
                "wqT": wqT_c,
                "wkT": wkT_c,
                "wvT": wvT_c,
                "woT": woT,
            }
        )
    return in_maps


def kernel(x, Wq, Wk, Wv, Wo):
    global LAST_EXEC_NS
    x = np.asarray(x, dtype=np.float32)
    Wq = np.asarray(Wq, dtype=np.float32)
    Wk = np.asarray(Wk, dtype=np.float32)
    Wv = np.asarray(Wv, dtype=np.float32)
    Wo = np.asarray(Wo, dtype=np.float32)
    in_maps = _host_in_maps(x, Wq, Wk, Wv, Wo)

    nc = _get_nc()
    res = run_bass_kernel_spmd(nc, in_maps, core_ids=list(range(NCORES)))
    LAST_EXEC_NS = getattr(res, "exec_time_ns", None)

    out = np.empty((B, L, D), dtype=np.float32)
    for c in range(NCORES):
        b, g = divmod(c, 4)
        out[b, 512 * g : 512 * (g + 1), :] = res.results[c]["y"]
    return out


# revision 17
# speedup vs baseline: 1.0733x; 1.0733x over previous
"""GQA causal attention block (B=2, L=2048, d_model=2048, 32 Q heads / 8 KV heads)
on 8 TRN2 NeuronCores.

Sharding: 8-way tensor parallel over heads, batch-paired. Core c owns q-heads
[4c, 4c+4) and kv-head c FOR BOTH batches.

Layouts:
  - qT (bf16): 4 tiles [128, L] = head-pair x batch: tile (t, b) holds local
    heads t (partitions 0:64) and t+2 (64:128) of batch b, feature-major.
  - kT (bf16): per batch a [128, L] tile with the kv head DUPLICATED in both
    partition halves, so heads 2,3 read lhsT/rhs at matching base 64.
  - v_aug (bf16): per key-block [128, 130]: cols 0:65 = batch0 (v | ones),
    65:130 = batch1 (v | ones).

Attention runs per head-pair (t, t+2 share a partition base, so consecutive
scores matmuls reuse the same kT stationary operand - the K=64 weight load
cannot be hidden otherwise). Scores^T -> exp (ScalarE, psum->bf16, causal
block masking) -> AV against V-with-ones-column (emits attn^T + softmax
denominator). Two AllToAll collectives (heads {0,1} then {2,3}) overlap with
the second half's attention and the first half's o_proj. o_proj (fp32r,
full Wo) accumulates even f-blocks after A2A#1 and odd after A2A#2; division
by the softmax denominator is fused in front via reciprocal + partition-
broadcast DMA + DVE multiply. The host just stacks output rows.
"""

import os
import sys
import math

os.environ.setdefault("MYCRO_LOCAL_CACHE", "1")
for _p in ("/opt/trn_rl_repo",):
    if os.path.isdir(_p) and _p not in sys.path:
        sys.path.insert(0, _p)

import numpy as np

import concourse.bass as bass
import concourse.bacc as bacc
import concourse.mybir as mybir
import concourse.tile as tile
from concourse.bass_utils import run_bass_kernel_spmd
from concourse.masks import make_identity, make_upper_triangular

F32 = mybir.dt.float32
F32R = mybir.dt.float32r
BF16 = mybir.dt.bfloat16
Exp = mybir.ActivationFunctionType.Exp

D = 2048          # d_model
L = 2048          # sequence length
DH = 64           # head dim
B = 2             # batch
NCORES = 8
NH_L = 4          # local q heads per core (per batch)
QF = NH_L * DH    # 256 local q features per batch
LC1 = 256         # phase-1 l-chunk (moving dim)
NLC1 = L // LC1   # 8
LC = 512          # attention l-tile
NLC = L // LC     # 4
NB = L // 128     # 16 key blocks of 128
SH = 130          # rows per A2A half-shard (2 heads x 64 + 2 denoms)
SCALE = 1.0 / math.sqrt(DH)

_CACHE = {}


def _mmr(nc, out, lhsT, rhs, **kw):
    nc.tensor.matmul(out, lhsT, rhs, **kw)


def _build_nc():
    nc = bacc.Bacc(
        "TRN2",
        target_bir_lowering=False,
        debug=False,
        enable_asserts=False,
        num_devices=NCORES,
    )
    xT0 = nc.dram_tensor("xT0", [D, L], F32, kind="ExternalInput")
    xT1 = nc.dram_tensor("xT1", [D, L], F32, kind="ExternalInput")
    wqT = nc.dram_tensor("wqT", [D, QF], F32, kind="ExternalInput")
    wkT = nc.dram_tensor("wkT", [D, DH], F32, kind="ExternalInput")
    wvT = nc.dram_tensor("wvT", [D, DH], F32, kind="ExternalInput")
    woT = nc.dram_tensor("woT", [D, D], F32, kind="ExternalInput")
    y = nc.dram_tensor("y", [LC, D], F32, kind="ExternalOutput")

    with tile.TileContext(nc) as tc:
        with tc.tile_pool(name="dram", bufs=1, space="DRAM") as dram:
            # one bounce pair per head-half (heads {0,1} / {2,3})
            bins = [
                dram.tile([NCORES * SH, LC], F32, name=f"bin{h}") for h in range(2)
            ]
            bouts = [
                dram.tile([NCORES * SH, LC], F32, name=f"bout{h}") for h in range(2)
            ]
            rdram = dram.tile([32, 512], F32, name="rdram")

            with tc.tile_pool(name="const", bufs=1) as const:
                ident = const.tile([128, 128], F32, name="ident")
                make_identity(nc, ident)
                tri = const.tile([128, 128], BF16, name="tri")
                make_upper_triangular(nc, tri, val=1.0)

                with tc.tile_pool(name="pers", bufs=1) as pers:
                    # qT[t][b]: heads t | t+2 of batch b
                    qT = [
                        [
                            pers.tile([128, L], BF16, name=f"qT{t}{b}")
                            for b in range(2)
                        ]
                        for t in range(2)
                    ]
                    kT = [pers.tile([128, L], BF16, name=f"kT{b}") for b in range(2)]
                    vaug = pers.tile([128, NB * 130], BF16, name="vaug")
                    va = vaug.rearrange("p (b c) -> p b c", c=130)
                    nc.gpsimd.memset(va[:, :, 64:65], 1.0)
                    nc.gpsimd.memset(va[:, :, 129:130], 1.0)

                    # wo pool: opened before phase 2 so DMAs overlap the
                    # attention compute, but EMITTED after phase 1 so they
                    # don't starve the x-column DMAs. Chunked by k-parity
                    # (even/odd f-blocks) to match the A2A-split o_proj order.
                    with tc.tile_pool(name="wo", bufs=1) as wop:
                        _phase1_qkv(
                            nc, tc, xT0, xT1, wqT, wkT, wvT, qT, kT, va, ident
                        )
                        wo_t = []  # wo_t[dc][parity] -> [128, 8*512]
                        for dc in range(4):
                            pair = []
                            for par in range(2):
                                w = wop.tile(
                                    [128, 8 * 512], F32R,
                                    name=f"wo{dc}{par}", tag=f"wo{par}",
                                )
                                src = woT[
                                    :, dc * 512 : (dc + 1) * 512
                                ].rearrange("(k two p) d -> p k two d", p=128, two=2)
                                nc.gpsimd.dma_start(
                                    w.rearrange("p (k d) -> p k d", d=512),
                                    src[:, :, par, :],
                                )
                                pair.append(w)
                            wo_t.append(pair)
                        with (
                            tc.tile_pool(name="p2s", bufs=1, space="PSUM") as scp,
                            tc.tile_pool(name="p2o", bufs=1, space="PSUM") as ovp,
                            tc.tile_pool(name="pbuf", bufs=1) as pbp,
                            tc.tile_pool(name="stg", bufs=2) as stp,
                        ):
                            for half in range(2):
                                _attn_half(
                                    nc, tc, qT, kT, va, bins[half], half,
                                    scp, ovp, pbp, stp, tri,
                                )
                                nc.gpsimd.collective_compute(
                                    "AllToAll",
                                    mybir.AluOpType.bypass,
                                    ins=[bins[half].opt()],
                                    outs=[bouts[half].opt()],
                                    replica_groups=[list(range(NCORES))],
                                )
                        _phase4_oproj(nc, tc, bouts, wo_t, rdram, y)
    nc.finalize()
    return nc


def _phase1_qkv(nc, tc, xT0, xT1, wqT, wkT, wvT, qT, kT, va, ident):
    """Projections. q: one M=128 fp32r matmul per (head-pair, batch, db).
    k/v: M=64 per batch at psum base 0; the partition-64 halves of kT are
    filled via a bf16 staging tile + SBUF->SBUF DMA."""
    with (
        tc.tile_pool(name="w1", bufs=1) as wpool,
        tc.tile_pool(name="xc", bufs=2) as xpool,
        tc.tile_pool(name="vt", bufs=2) as vtpool,
        tc.tile_pool(name="p1", bufs=1, space="PSUM") as p1,
    ):
        wq_sb = wpool.tile([128, 16 * QF], F32R, name="wq_sb")
        wk_sb = wpool.tile([128, 16 * DH], F32R, name="wk_sb")
        wv_sb = wpool.tile([128, 16 * DH], F32R, name="wv_sb")
        for w_sb, w_dram, fw in ((wq_sb, wqT, QF), (wk_sb, wkT, DH), (wv_sb, wvT, DH)):
            nc.gpsimd.dma_start(
                w_sb.rearrange("p (b f) -> p b f", f=fw),
                w_dram.rearrange("(b p) f -> p b f", p=128),
            )

        for lc in range(NLC1):
            x0 = xpool.tile([128, 16 * LC1], F32R, name="x0", tag="x0")
            x1 = xpool.tile([128, 16 * LC1], F32R, name="x1", tag="x1")
            for xt, xdram in ((x0, xT0), (x1, xT1)):
                nc.gpsimd.dma_start(
                    xt.rearrange("p (b l) -> p b l", l=LC1),
                    xdram[:, lc * LC1 : (lc + 1) * LC1].rearrange(
                        "(b p) l -> p b l", p=128
                    ),
                )
            cols = slice(lc * LC1, (lc + 1) * LC1)
            for grp in range(2):
                aq = [
                    p1.tile([128, LC1], F32, name=f"aq{b}", tag=f"aq{b}")
                    for b in range(2)
                ]
                akv = [
                    p1.tile([64, LC1], F32, name=f"akv{b}", tag=f"akv{b}")
                    for b in range(2)
                ]
                for db in range(16):
                    rx = (
                        x0[:, db * LC1 : (db + 1) * LC1],
                        x1[:, db * LC1 : (db + 1) * LC1],
                    )
                    st = dict(start=(db == 0), stop=(db == 15))
                    wjp = wq_sb[:, db * QF + grp * 128 : db * QF + (grp + 1) * 128]
                    wkv = wk_sb if grp == 0 else wv_sb
                    wb = wkv[:, db * DH : (db + 1) * DH]
                    for b in range(2):
                        _mmr(nc, aq[b][:, :], wjp, rx[b], **st)
                        _mmr(nc, akv[b][:, :], wb, rx[b], **st)
                for b in range(2):
                    nc.scalar.copy(qT[grp][b][:, cols], aq[b][:, :])
                if grp == 0:
                    for b in range(2):
                        nc.scalar.copy(kT[b][0:64, cols], akv[b][:, :])
                        stk = vtpool.tile([64, LC1], BF16, name="stk", tag=f"stk{b}")
                        nc.scalar.copy(stk[:, :], akv[b][:, :])
                        nc.sync.dma_start(kT[b][64:128, cols], stk[:, :])
                else:
                    vt0 = vtpool.tile([64, LC1], F32, name="vt0", tag="vt0")
                    vt1 = vtpool.tile([64, LC1], F32, name="vt1", tag="vt1")
                    nc.scalar.copy(vt0[:, :], akv[0][:, :])
                    nc.scalar.copy(vt1[:, :], akv[1][:, :])
                    for s in range(LC1 // 128):
                        beta = (lc * LC1) // 128 + s
                        tp = p1.tile([128, 128], F32, name="tp", tag="tp", bufs=2)
                        nc.tensor.matmul(
                            tp[:, 0:64],
                            vt0[:, s * 128 : (s + 1) * 128],
                            ident[0:64, 0:64],
                            is_transpose=True,
                        )
                        nc.tensor.matmul(
                            tp[:, 64:128],
                            vt1[:, s * 128 : (s + 1) * 128],
                            ident[0:64, 0:64],
                            is_transpose=True,
                            skip_group_check=True,
                        )
                        nc.scalar.copy(va[:, beta, 0:64], tp[:, 0:64])
                        nc.scalar.copy(va[:, beta, 65:129], tp[:, 64:128])


def _attn_half(nc, tc, qT, kT, va, bin_, half, scp, ovp, pbp, stp, tri):
    """Attention for local heads {2*half, 2*half+1}: half 0 -> heads 0,1
    (tiles t=0,1 at partitions 0:64), half 1 -> heads 2,3 (at 64:128).
    Both heads of a half share the kT partition base, and the scores
    emission groups the two heads per kT block so the un-hideable K=64
    weight load is amortized over two matmuls."""
    po = 64 * half
    for tau in range(NLC):
        nb = 4 * tau + 4
        lcols = slice(tau * LC, (tau + 1) * LC)
        P = [
            [
                pbp.tile([128, NB * 512], BF16, name=f"p{u}{b}", tag=f"p{u}{b}")
                for b in range(2)
            ]
            for u in range(2)
        ]
        q = [[qT[u][b][po : po + 64, lcols] for b in range(2)] for u in range(2)]

        # full strips: per batch, per key-block pair; both heads back to back
        # on the same stationary kT block
        for b in range(2):
            for b0 in range(0, 4 * tau, 2):
                scs = [
                    scp.tile([128, 1024], F32, name=f"sc{u}", tag=f"sc{u}")
                    for u in range(2)
                ]
                k0 = kT[b][po : po + 64, b0 * 128 : (b0 + 1) * 128]
                k1 = kT[b][po : po + 64, (b0 + 1) * 128 : (b0 + 2) * 128]
                for u in range(2):
                    nc.tensor.matmul(scs[u][:, 0:512], k0, q[u][b])
                for u in range(2):
                    nc.tensor.matmul(scs[u][:, 512:1024], k1, q[u][b])
                for u in range(2):
                    nc.scalar.activation(
                        P[u][b][:, b0 * 512 : (b0 + 2) * 512],
                        scs[u][:, 0:1024],
                        Exp,
                    )
            # diagonal strips: masking via DVE (memset + tri multiply) so the
            # gpsimd queue stays free for the collectives
            for dj in range(4):
                beta = 4 * tau + dj
                kb = kT[b][po : po + 64, beta * 128 : (beta + 1) * 128]
                scs = [
                    scp.tile([128, 1024], F32, name=f"sc{u}", tag=f"sc{u}")
                    for u in range(2)
                ]
                for u in range(2):
                    nc.tensor.matmul(scs[u][:, 0:512], kb, q[u][b])
                for u in range(2):
                    base = beta * 512
                    if dj > 0:
                        nc.vector.memset(P[u][b][:, base : base + dj * 128], 0.0)
                    nc.scalar.activation(
                        P[u][b][:, base + dj * 128 : base + 512],
                        scs[u][:, dj * 128 : 512],
                        Exp,
                    )
                    dg = P[u][b][:, base + dj * 128 : base + (dj + 1) * 128]
                    nc.vector.tensor_mul(dg, dg, tri[:, :])

        # AV: 4 interleaved accumulation chains (u x b)
        ov = [
            [
                ovp.tile([128, 512], F32, name=f"o{u}{b}", tag=f"o{u}{b}")
                for b in range(2)
            ]
            for u in range(2)
        ]
        for bk in range(nb):
            st = dict(start=(bk == 0), stop=(bk == nb - 1))
            for u in range(2):
                for b in range(2):
                    nc.tensor.matmul(
                        ov[u][b][0:65, :],
                        va[:, bk, 65 * b : 65 * b + 65],
                        P[u][b][:, bk * 512 : (bk + 1) * 512],
                        **st,
                    )
        # stage to bounce: shard (4*b + tau): rows 64*u attn, 128+u denom
        for u in range(2):
            st1 = stp.tile([128, 512], F32, name="st1", tag="st1")
            nc.scalar.copy(st1[0:64, :], ov[u][0][0:64, :])
            nc.scalar.copy(st1[64:128, :], ov[u][1][0:64, :])
            ds = stp.tile([128, 1024], F32, name="ds", tag="ds")
            nc.vector.tensor_copy(ds[64:65, 0:512], ov[u][0][64:65, :])
            nc.vector.tensor_copy(ds[64:65, 512:1024], ov[u][1][64:65, :])
            for b in range(2):
                sh = SH * (4 * b + tau)
                nc.sync.dma_start(
                    bin_[sh + 64 * u : sh + 64 * (u + 1), :],
                    st1[64 * b : 64 * (b + 1), :],
                )
                nc.sync.dma_start(
                    bin_[sh + 128 + u : sh + 128 + u + 1, :],
                    ds[64:65, 512 * b : 512 * b + 512],
                )


def _phase4_oproj(nc, tc, bouts, wo_t, rdram, y):
    """Normalize and o_proj. f-block k holds global heads 2k,2k+1 =
    bounce[(2k)%4//2][shard k//2] rows 0:128; even k uses bout0, odd bout1."""
    with (
        tc.tile_pool(name="an", bufs=1, side="right") as anp,
        tc.tile_pool(name="den", bufs=1, side="right") as denp,
        tc.tile_pool(name="ysb", bufs=2, side="right") as yp,
        tc.tile_pool(name="p4y", bufs=4, space="PSUM") as eyp,
    ):
        # reciprocals per half: rdram rows = global head id
        for half in range(2):
            dall = denp.tile([16, 512], F32, name=f"dall{half}", tag="dall", bufs=2)
            for c in range(NCORES):
                nc.sync.dma_start(
                    dall[2 * c : 2 * (c + 1), :],
                    bouts[half][SH * c + 128 : SH * c + 130, :],
                )
            rall = denp.tile([16, 512], F32, name=f"rall{half}", tag="rall", bufs=2)
            nc.vector.reciprocal(rall[:, :], dall[:, :])
            # rdram row layout: 16*half + 2*c + u  (head h = 4c + 2*half + u)
            nc.sync.dma_start(rdram[16 * half : 16 * half + 16, :], rall[:, :])

        ans = [None] * 16
        for k in sorted(range(16), key=lambda k: k % 2):
            half = k % 2
            c = k // 2
            au = anp.tile([128, 512], F32, name=f"au{k}", tag="au", bufs=4)
            nc.sync.dma_start(
                au[:, :], bouts[half][SH * c : SH * c + 128, :]
            )
            # heads 2k, 2k+1 -> rdram rows 16*half + 2*c (+1)
            rA = 16 * half + 2 * c
            dv = anp.tile([128, 512], F32, name="dv", tag="dv", bufs=4)
            nc.sync.dma_start(
                dv[0:64, :], rdram[rA : rA + 1, :].partition_broadcast(64)
            )
            nc.sync.dma_start(
                dv[64:128, :], rdram[rA + 1 : rA + 2, :].partition_broadcast(64)
            )
            an = anp.tile([128, 512], F32R, name=f"an{k}", tag=f"an{k}")
            nc.vector.tensor_mul(an[:, :], au[:, :], dv[:, :])
            ans[k] = an

        korder = sorted(range(16), key=lambda k: k % 2)
        for dc in range(4):
            for m in range(4):
                yps = eyp.tile([128, 512], F32, name="yps", tag="yps")
                for i, k in enumerate(korder):
                    _mmr(
                        nc, yps[:, :],
                        ans[k][:, m * 128 : (m + 1) * 128],
                        wo_t[dc][k % 2][:, (k // 2) * 512 : (k // 2 + 1) * 512],
                        start=(i == 0), stop=(i == 15),
                    )
                ysb = yp.tile([128, 512], F32, name="ysb", tag="ysb")
                nc.scalar.copy(ysb[:, :], yps[:, :])
                nc.sync.dma_start(
                    y[m * 128 : (m + 1) * 128, dc * 512 : (dc + 1) * 512], ysb[:, :]
                )


def _get_nc():
    if "nc" not in _CACHE:
        _CACHE["nc"] = _build_nc()
    return _CACHE["nc"]


LAST_EXEC_NS = None


def _host_in_maps(x, Wq, Wk, Wv, Wo):
    xT0 = np.ascontiguousarray(x[0].T)
    xT1 = np.ascontiguousarray(x[1].T)
    woT = np.ascontiguousarray(Wo.T)

    in_maps = []
    for c in range(NCORES):
        # local head order in wq columns: pair-tile layout (t | t+2):
        # tile 0 = heads 0,2 ; tile 1 = heads 1,3  (within this core)
        rows = []
        for t in range(2):
            for u in range(2):
                h = QF * c // 64 + t + 2 * u  # 4c + t + 2u
                rows.append(Wq[64 * h : 64 * (h + 1), :])
        # order per 128-block: block grp holds heads (grp, grp+2)
        wq_rows = np.concatenate(
            [rows[0], rows[1], rows[2], rows[3]], axis=0
        )
        wqT_c = np.ascontiguousarray((SCALE * wq_rows).T)
        wkT_c = np.ascontiguousarray(Wk[DH * c : DH * (c + 1), :].T)
        wvT_c = np.ascontiguousarray(Wv[DH * c : DH * (c + 1), :].T)
        in_maps.append(
            {
                "xT0": xT0,
                "xT1": xT1,
                "wqT": wqT_c,
                "wkT": wkT_c,
                "wvT": wvT_c,
                "woT": woT,
            }
        )
    return in_maps


def kernel(x, Wq, Wk, Wv, Wo):
    global LAST_EXEC_NS
    x = np.asarray(x, dtype=np.float32)
    Wq = np.asarray(Wq, dtype=np.float32)
    Wk = np.asarray(Wk, dtype=np.float32)
    Wv = np.asarray(Wv, dtype=np.float32)
    Wo = np.asarray(Wo, dtype=np.float32)
    in_maps = _host_in_maps(x, Wq, Wk, Wv, Wo)

    nc = _get_nc()
    res = run_bass_kernel_spmd(nc, in_maps, core_ids=list(range(NCORES)))
    LAST_EXEC_NS = getattr(res, "exec_time_ns", None)

    out = np.empty((B, L, D), dtype=np.float32)
    for c in range(NCORES):
        b, g = divmod(c, 4)
        out[b, 512 * g : 512 * (g + 1), :] = res.results[c]["y"]
    return out
